# revision 34
# baseline (speedup 1.0000x reference)
"""AsymmetricEMA Trainium2 kernel (8 NeuronCores, Bass/Tile).

Reference recurrence: y_0 = x_0; y_t = a*y_{t-1} + (1-a)*x_t with
a = 0.99 if y_{t-1} > x_t else 0.5.  Equivalently (exactly):

    y_t = max(0.99*(y_{t-1}-x_t), 0.5*(y_{t-1}-x_t)) + x_t

**d-space formulation**: with d_t = y_t - x_t and delta_t = x_{t-1} - x_t
(precomputed on the host from the f32 input during the layout gather):

    d_t = max(0.99*u, 0.5*u),   u = d_{t-1} + delta_t ;   y_t = d_t + x_t

The device ships delta in, d out (same bytes as x/y), and the host adds x
back in f32 during the scatter.  The win: the DVE step body is 4 ALU ops
(add, mul, mul, max), which fits TWICE in the 8-stage DVE datapath -- so a
hand-written 2X_1PORT uop program (2 packed fp16/cycle) runs the whole
recurrence at 2 elem/cycle instead of 1 (validated exact on HW).

Time axis split into C=16 chunks of L=256 processed in parallel, each
warmed up with W=96 extra steps started from d := 0 at chunk_start - W
(contraction makes the warmed-up state accurate to ~1e-3 rel-norm).

Layout: host pre-gathers delta into round-major SBUF streaming order with
[C, G] column layout (chunk-major, group-minor) so the warmup chunk-shift
is -G elements (32B, 4B-aligned) and every DVE AP qualifies for 2x mode.
Each refill round is ONE contiguous 2 MiB DMA; drains are merged into one
DMA per contiguous Y-buffer run (3 DMAs for NBUFY=6), each targeting its
own partition-major-contiguous DRAM tensor (og0..og2).

DMA schedule: reads and writes are direction-phased (all refills precede
all drains on the single qSP HWDGE FIFO; refills that reuse a stream
buffer are emitted right after the body that frees it): measured per-core
HBM rates are 483 GB/s read-only, 397 GB/s write-only, but only ~371 GB/s
combined when directions interleave -- phase separation avoids the
read/write turnaround tax.  NBUFY=6 d-state rings let the DVE run ahead
of the trailing drain phase.

Sharding: batch (16) across 8 cores, 2 batches/core, pure data parallel.
"""
import os
import numpy as np
import orjson

# ---------------------------------------------------------------------------
# container workaround: this walrus build allows ONE sync-wait per
# instruction; hoist extras onto NoOps inserted before (same engine =>
# same order => identical sync semantics).
# ---------------------------------------------------------------------------
from concourse import bass as _bass

_MAX_WAITS = 1
_orig_to_json_bytes = _bass.Bass.to_json_bytes


def _split_waits_json(data: bytes) -> bytes:
    j = orjson.loads(data)
    n = [0]
    changed = False
    for fn in j.get("functions", []):
        for bb in fn.get("blocks", []):
            out = []
            for inst in bb.get("instructions", []):
                si = inst.get("sync_info")
                if si:
                    waits = si.get("on_wait") or []
                    if len(waits) > _MAX_WAITS:
                        changed = True
                        for w in waits[:-_MAX_WAITS]:
                            n[0] += 1
                            out.append({
                                "debug": inst.get("debug", 0),
                                "engine": inst["engine"],
                                "ins": [], "outs": [],
                                "name": f"I-waitsplit-{n[0]}",
                                "opcode": "NoOp",
                                "sync_info": {"on_update": [],
                                              "on_wait": [w]},
                            })
                        si["on_wait"] = waits[-_MAX_WAITS:]
                out.append(inst)
            bb["instructions"] = out
    return orjson.dumps(j) if changed else data


def _to_json_bytes_patched(self, *a, **k):
    return _split_waits_json(_orig_to_json_bytes(self, *a, **k))


_bass.Bass.to_json_bytes = _to_json_bytes_patched

from concourse import bass, mybir  # noqa: E402
from concourse.tile import TileContext  # noqa: E402
from concourse.bass_utils import run_bass_kernel_spmd  # noqa: E402

F16 = mybir.dt.float16
AF, AR = 0.99, 0.5

# ---------------------------------------------------------------------------
# custom DVE op: out = max((in0+in1)*C0, (in0+in1)*C1), with a hand-written
# 2X_1PORT uop program (el0 on datapath blocks 0-3, el1 on blocks 4-7).
# ---------------------------------------------------------------------------
_EMA_OP = [None]


def _get_ema_d_op():
    if _EMA_OP[0] is not None:
        return _EMA_OP[0]
    from concourse.dve_spec import Spec, Src0, Src1, C0, C1, maxx, lower
    from concourse.dve_uop import (DveOpSpec, UopConfig, AluOp, AluInp,
                                   DelayInp, InpSel, OutSel, OutPath,
                                   Trigger)
    from concourse import dve_ops
    from concourse.dve_ops import DveOp, OPS, _COMPILE_CACHE

    def _ref(in0, in1, s0, s1, imm2):
        u = (in0 + in1).astype(np.float32)
        return np.maximum(u * np.float32(s0), u * np.float32(s1)).astype(
            np.float32)

    u = Src0 + Src1
    spec = Spec(body=maxx(u * C0, u * C1), reference=_ref)
    uops_1x = lower(spec, ver="v3")

    A, D = AluInp, DelayInp
    u2 = UopConfig()
    u2.enable_input(InpSel.SRC_0, 1)      # chain0 at blk0
    u2.enable_input(InpSel.SRC_1, 2)      # chain1
    u2.enable_input(InpSel.CONST_0, 3)    # chain2
    u2.enable_input(InpSel.CONST_1, 4)    # chain3
    u2.enable_input(InpSel.SRC_0_HI, 5)   # chain4
    u2.enable_input(InpSel.SRC_1_HI, 6)   # chain5
    u2.require_inp0 = 1
    u2.require_inp1 = 1
    u2.trigger = (Trigger.SRC_TENSOR_DONE, Trigger.NONE, Trigger.NONE)
    b = u2.datapath_config
    # el0 (LO halves)
    b[0].enable_alu(AluOp.ADD, A.PREV_DELAY_0, A.PREV_DELAY_1)         # u0
    b[0].pass_through_delay(2, 3, 4, 5)
    b[1].enable_alu(AluOp.MULTIPLY, A.PREV_ALU_OUT, A.PREV_DELAY_2)    # m1_0
    b[1].enable_delay_from_src(D.PREV_ALU_OUT, 0)                      # u0
    b[1].pass_through_delay(2, 3, 4, 5)
    b[2].enable_alu(AluOp.MULTIPLY, A.PREV_DELAY_0, A.PREV_DELAY_3)    # m2_0
    b[2].enable_delay_from_src(D.PREV_ALU_OUT, 0)                      # m1_0
    b[2].pass_through_delay(2, 3, 4, 5)
    b[3].enable_alu(AluOp.MAX, A.PREV_DELAY_0, A.PREV_ALU_OUT)         # r0
    b[3].pass_through_delay(2, 3, 4, 5)
    # el1 (HI halves)
    b[4].enable_alu(AluOp.ADD, A.PREV_DELAY_4, A.PREV_DELAY_5)         # u1
    b[4].enable_delay_from_src(D.PREV_ALU_OUT, 0)                      # r0
    b[4].pass_through_delay(2, 3)
    b[5].enable_alu(AluOp.MULTIPLY, A.PREV_ALU_OUT, A.PREV_DELAY_2)    # m1_1
    b[5].enable_delay_from_src(D.PREV_ALU_OUT, 1)                      # u1
    b[5].pass_through_delay(0, 3)
    b[6].enable_alu(AluOp.MULTIPLY, A.PREV_DELAY_1, A.PREV_DELAY_3)    # m2_1
    b[6].enable_delay_from_src(D.PREV_ALU_OUT, 1)                      # m1_1
    b[6].pass_through_delay(0)
    b[7].enable_alu(AluOp.MAX, A.PREV_DELAY_1, A.PREV_ALU_OUT)         # r1
    b[7].pass_through_delay(0)
    u2.enable_output(OutSel.DELAY_0, OutPath.WR0_LO)   # r0
    u2.enable_output(OutSel.ALU_OUT, OutPath.WR0_HI)   # r1
    u2.validate("v3")

    name = "EMA_D2X_ANT"
    op = DveOp(name, spec, subdim=False, uops_sha={})
    if name not in dve_ops._SUB_OPCODE_FOR_NAME:
        OPS.append(op)
        dve_ops._SUB_OPCODE_FOR_NAME[name] = (
            dve_ops._CUSTOM_DVE_ROW_BASE + len(OPS) - 1)
    dve_ops.CUSTOM_DVE_SPECS[name] = spec
    row = dve_ops.get_dve_sub_opcode(name)
    compiled = DveOpSpec(name=name, opcode=row, uops=uops_1x,
                         uops_2x=[u2], rd1_en=True, perf_max=1)
    _COMPILE_CACHE[(name, "v3")] = compiled
    _EMA_OP[0] = op
    return op


# default kernel geometry (hardcoded for the 16x4096x1024 problem)
L, W, BLK = 256, 96, 32
NBUFY = int(os.environ.get("EMA_NBUFY", "6"))
NSTREAM_ENV = int(os.environ.get("EMA_NSTREAM", "3"))
DRAIN_MODE = os.environ.get("EMA_DRAIN", "late")  # late | interleave
# drain merge groups (rounds): maximal runs of rounds whose Y buffers
# (NP + q) % NBUFY are contiguous, so each group drains in ONE DMA.
def _dgroups(NR=8, NP=3, nbufy=None):
    nbufy = nbufy or NBUFY
    out, q0 = [], 0
    while q0 < NR:
        b0 = (NP + q0) % nbufy
        n = min(nbufy - b0, NR - q0)
        out.append((q0, q0 + n))
        q0 += n
    return out

DGROUPS = _dgroups()
if os.environ.get("EMA_SPLITD0", "1") == "1":
    # split the leading drain group so the write phase's first DMA only
    # waits on DVE round 0 (earlier leading edge)
    g0, g1 = DGROUPS[0]
    if g1 - g0 > 1:
        DGROUPS = [(g0, g0 + 1), (g0 + 1, g1)] + DGROUPS[1:]


# ---------------------------------------------------------------------------
# per-core SPMD program
# ---------------------------------------------------------------------------
def _build(B_PER_CORE=2, T=4096, NCH=1024, reps=1, mode="normal",
           bodies_per_rep=1):
    CBLK = NCH // 128          # channel blocks of 128 partitions
    G = B_PER_CORE * CBLK      # partition-groups (inner dim of a column)
    C = T // L                 # parallel time chunks (outer dim of a column)
    NR = L // BLK              # body column rounds (delta is read once)
    NP = W // BLK              # warmup rounds == pinned tail rounds
    NSTREAM = NSTREAM_ENV      # streaming delta buffers (rounds 0..NR-NP-1)
    NBUFX = NSTREAM + NP
    CW = G * C                 # one column's elements per partition
    FB = BLK * CW              # one buffer, flat
    assert W % BLK == 0 and L % BLK == 0 and NP + NSTREAM <= NR
    ema_op = _get_ema_d_op()

    nc = bass.Bass(num_devices=8)
    # x: round-major (each refill round = one fully contiguous 2 MiB DRAM
    # region -> maximal descriptors).  out: one DRAM tensor per drain merge
    # group, partition-major within the group, so each merged drain is one
    # DMA with large contiguous per-partition runs.
    x_ext = nc.declare_dram_parameter("x", [NR, 128, FB], F16,
                                      isOutput=False)
    og_ext = [nc.declare_dram_parameter(f"og{i}", [128, (g1 - g0) * FB],
                                        F16, isOutput=True)
              for i, (g0, g1) in enumerate(DGROUPS)]

    with TileContext(nc) as tc:
        with tc.tile_pool(name="rings", bufs=1) as rpool:
            # column layout [C, G]: chunk-major, group-minor, so the warmup
            # chunk-shift is -G elements (4B-aligned -> 2x eligible).
            X = rpool.tile([128, NBUFX, BLK, C, G], F16)   # delta buffers
            Y = rpool.tile([128, NBUFY, BLK, C, G], F16)   # d-state buffers
            XA = X.rearrange("p n a b c -> p (n a b c)")

            def xbuf(q):
                # body round q -> delta buffer index
                return q % NSTREAM if q < NR - NP else NSTREAM + (q - (NR - NP))

            def refill(q):
                nc.sync.dma_start(
                    out=X[:, xbuf(q)].rearrange("p a b c -> p (a b c)"),
                    in_=x_ext[q])

            deng = nc.scalar if os.environ.get("EMA_DRAINQ") == "act" \
                else nc.sync

            def drain_group(gi):
                # rounds g0..g1-1 from Y bufs (NP+g0)%NBUFY.. in ONE DMA
                g0, g1 = DGROUPS[gi]
                b0 = (NP + g0) % NBUFY
                assert b0 + (g1 - g0) <= NBUFY
                deng.dma_start(
                    out=og_ext[gi][:],
                    in_=Y[:, b0:b0 + (g1 - g0)].rearrange(
                        "p n a b c -> p (n a b c)"))

            def out_slice(qd):
                # round qd's slice of its group tensor (analysis modes)
                for gi, (g0, g1) in enumerate(DGROUPS):
                    if g0 <= qd < g1:
                        return og_ext[gi][:, (qd - g0) * FB:(qd - g0 + 1) * FB]
                raise AssertionError(qd)

            def drain(qd):
                deng.dma_start(
                    out=out_slice(qd),
                    in_=Y[:, (NP + qd) % NBUFY].rearrange(
                        "p a b c -> p (a b c)"))

            def flat(tile, q):
                return tile[:, q].rearrange("p a b c -> p (a b c)")

            def ema(out, in0, in1, s0=AF, s1=AR):
                binst = nc.vector._custom_dve(ema_op, out=out, in0=in0,
                                              in1=in1, s0=s0, s1=s1)
                binst.ins.perf_max = int(os.environ.get("EMA_PERF", "1"))
                return binst

            def ema0(out, src):
                # d := 0 (s0=s1=0 => max(0*u, 0*u) = 0; src just needs to be
                # a finite readable tile -- delta data, never junk SBUF).
                return ema(out, src, src, s0=0.0, s1=0.0)

            def warmup_buffer(wb):
                """Warmup columns [wb*BLK, wb*BLK+BLK): every chunk c reads
                chunk c-1's tail delta from pinned round NR-NP+wb via a
                G-element-shifted flat AP (chunk 0 lanes read junk; their
                state is re-initialized at the i==0 boundary)."""
                pb = NSTREAM + wb          # pinned delta buffer
                off = pb * FB              # flat offset of that buffer
                YF = flat(Y, wb % NBUFY)
                if mode == "nodep":
                    ema(YF[:, 0:FB], XA[:, off:off + FB], XA[:, off:off + FB])
                    return
                yb = wb % NBUFY
                if wb == 0:
                    # warmup start: d := 0 for every chunk, one instruction
                    ema0(Y[:, yb, 0], X[:, pb, 0])
                else:
                    ema(Y[:, yb, 0, 1:, :], Y[:, (wb - 1) % NBUFY, BLK - 1, 1:, :],
                        X[:, pb, 0, 0:C - 1, :])
                    ema0(Y[:, yb, 0, 0:1, :], X[:, pb, 0, 0:1, :])
                ema(YF[:, CW:FB], YF[:, 0:FB - CW],
                    XA[:, off + CW - G:off + FB - G])

            def body_buffer(qb):
                """Body columns [qb*BLK, qb*BLK+BLK) in two (three at the
                i==0 boundary) DVE instructions.  Instr B covers BLK-1
                columns in ONE instruction whose in0 is its own out shifted
                one column back -- elements stream in AP order, so each
                read lands CW elements after the write it depends on."""
                q = (NP + qb) % NBUFY
                YF, XF = flat(Y, q), flat(X, xbuf(qb))
                if mode == "nodep":
                    ema(YF[:, 0:FB], XF[:, 0:FB], XF[:, 0:FB])
                    return
                if qb == 0:
                    # i==0: chunks 1.. continue from warmup; chunk 0: d := 0
                    ema(Y[:, q, 0, 1:, :], Y[:, (NP - 1) % NBUFY, BLK - 1, 1:, :],
                        X[:, xbuf(0), 0, 1:, :])
                    ema0(Y[:, q, 0, 0:1, :], X[:, xbuf(0), 0, 0:1, :])
                else:
                    YP = flat(Y, (NP + qb - 1) % NBUFY)
                    ema(YF[:, 0:CW], YP[:, FB - CW:FB], XF[:, 0:CW])
                ema(YF[:, CW:FB], YF[:, 0:FB - CW], XF[:, CW:FB])

            def body():
                if mode == "dmaonly":
                    for q in range(NR):
                        refill(q)
                    return
                if mode in ("dmaio", "dmaio2"):
                    eng = nc.scalar if mode == "dmaio2" else nc.sync
                    for q in range(NR):
                        refill(q)
                        eng.dma_start(
                            out=out_slice(q),
                            in_=X[:, xbuf(q)].rearrange("p a b c -> p (a b c)"))
                    return
                if mode == "drainonly":
                    for q in range(NR):
                        nc.sync.dma_start(
                            out=out_slice(q),
                            in_=X[:, q % NBUFX].rearrange("p a b c -> p (a b c)"))
                    return
                if mode == "dmaphase":
                    for q in range(NR):
                        nc.sync.dma_start(
                            out=X[:, q % NSTREAM].rearrange("p a b c -> p (a b c)"),
                            in_=x_ext[q])
                    for q in range(NR):
                        nc.sync.dma_start(
                            out=out_slice(q),
                            in_=X[:, NSTREAM + q % NP].rearrange("p a b c -> p (a b c)"))
                    return
                if mode == "debug":
                    # X preloaded (below); run N warmups + M bodies, then
                    # drain warmup bufs to out rounds 0..NP-1 and body bufs
                    # to rounds NP..  (EMA_DBG="<nw>,<nb>")
                    nw, nb = map(int, os.environ.get("EMA_DBG", "3,0").split(","))
                    for wb in range(nw):
                        warmup_buffer(wb)
                    for qb in range(nb):
                        body_buffer(qb)
                    for wb in range(nw):
                        nc.sync.dma_start(
                            out=out_slice(wb),
                            in_=Y[:, wb % NBUFY].rearrange("p a b c -> p (a b c)"))
                    for qb in range(nb):
                        nc.sync.dma_start(
                            out=out_slice(NP + qb),
                            in_=Y[:, (NP + qb) % NBUFY].rearrange("p a b c -> p (a b c)"))
                    return
                do_dma = mode != "dveonly"
                do_refill = do_dma and mode != "norefill"
                do_drain = do_dma and mode != "nodrain"
                if do_refill:
                    # pinned tail first (warmup input), then the stream
                    # prefill.  Later stream refills are emitted right after
                    # the body that frees their buffer (program order defines
                    # both the Tile data binding and the qSP FIFO order).
                    for q in range(NR - NP, NR):
                        refill(q)
                    for q in range(NSTREAM):
                        refill(q)
                for wb in range(NP):
                    warmup_buffer(wb)
                # merged drains: group gi is emitted right after the body
                # whose round completes it (so its DVE wait can pass), and
                # always after all refill emissions (direction phasing on
                # the qSP FIFO).
                drain_after = {DGROUPS[gi][1] - 1: gi
                               for gi in range(len(DGROUPS))}
                for qb in range(NR):
                    body_buffer(qb)
                    if do_drain and DRAIN_MODE == "interleave":
                        drain(qb)
                    nq = NSTREAM + qb
                    if do_refill and nq < NR - NP:
                        refill(nq)
                    if do_drain and DRAIN_MODE == "late" and qb in drain_after:
                        drain_group(drain_after[qb])

            if mode in ("dveonly", "drainonly", "dmaphase", "debug", "norefill"):
                for q in range(NBUFX):
                    rnd = q if q < NSTREAM else NR - NP + (q - NSTREAM)
                    nc.sync.dma_start(
                        out=X[:, q].rearrange("p a b c -> p (a b c)"),
                        in_=x_ext[rnd])
            if reps == 1 and bodies_per_rep > 1:
                # unrolled replication with a cross-core barrier after each
                # body: keeps the 8 cores phase-aligned (as in a real
                # single-shot launch) for faithful per-body timing.
                # EMA_NBARRIER=2 doubles the barrier for barrier-cost calib.
                nb = int(os.environ.get("EMA_NBARRIER", "0"))
                for _ in range(bodies_per_rep):
                    body()
                    for _ in range(nb):
                        nc.all_core_barrier()
            elif reps == 1:
                body()
            else:
                # timing-only replication: repeat the identical program in a
                # hardware loop so exec time ~= reps * body.  NOTE: across
                # reps the 8 cores drift out of phase, so their read/write
                # DMA phases mix at the HBM level; the per-body estimate is
                # therefore a CONSERVATIVE (upper) bound on the single-shot
                # time, where all cores launch in lockstep and the
                # direction-phased DMA schedule avoids r/w turnaround.
                with tc.For_i(0, reps):
                    for _ in range(bodies_per_rep):
                        body()

    mybir.codegen_inst_isa_subclasses(nc)
    return nc


_NC_CACHE = [None]


def _gather_inputs(x):
    """x: [B, T, NCH] f32 -> list of per-core round-major fp16 delta arrays.

    delta_t = x_{t-1} - x_t (delta_0 = 0, never read).  Round-major with
    [C, G] columns: xdev[q][p][j][c][(b,k)] = delta[b, k*128+p, c*L + q*BLK + j].
    """
    B, T, NCH = x.shape
    n_cores, bpc = 8, B // 8
    CBLK = NCH // 128
    C = T // L
    delta = np.empty_like(x)
    delta[:, 0] = 0.0
    np.subtract(x[:, :-1], x[:, 1:], out=delta[:, 1:])
    dt_ = np.swapaxes(delta, 1, 2).astype(np.float16)   # [B, NCH, T]
    xg = dt_.reshape(B, CBLK, 128, C, L // BLK, BLK)
    # xg: [b, k, p, c, q, j] -> [q, p, j, c, b, k] (round-major DRAM)
    xg = np.ascontiguousarray(xg.transpose(4, 2, 5, 3, 0, 1))
    ins = []
    for kc in range(n_cores):
        blk = xg[:, :, :, :, bpc * kc:bpc * (kc + 1)]  # [NR,128,BLK,C,bpc,CBLK]
        ins.append(np.ascontiguousarray(
            blk.reshape(blk.shape[0], 128, -1)))
    return ins


def _scatter_outputs(outs, x):
    """outs: per-core dict of drain-group arrays og{i} [128, ng*FB] fp16
    d-values -> y = d + x f32."""
    B, T, NCH = x.shape
    n_cores, bpc = 8, B // 8
    CBLK = NCH // 128
    C = T // L
    NRD = L // BLK
    y = np.empty((B, NCH, T), np.float32)
    for kc in range(n_cores):
        # reassemble [NRD, 128, BLK, C, bpc, CBLK] from group tensors
        parts = []
        for gi, (g0, g1) in enumerate(DGROUPS):
            a = outs[kc][f"og{gi}"].reshape(128, g1 - g0, BLK, C, bpc, CBLK)
            parts.append(a.transpose(1, 0, 2, 3, 4, 5))
        o = np.concatenate(parts, axis=0)
        # -> [bpc, CBLK, 128, C, NRD, BLK]
        o = o.transpose(4, 5, 1, 3, 0, 2).astype(np.float32)
        o = o.reshape(bpc, NCH, C, L)
        y[bpc * kc:bpc * (kc + 1)] = o.reshape(bpc, NCH, T)
    yout = np.ascontiguousarray(np.swapaxes(y, 1, 2))
    yout += x
    return yout


def kernel(x: np.ndarray) -> np.ndarray:
    x = np.asarray(x, dtype=np.float32)
    B, T, NCH = x.shape  # (16, 4096, 1024)
    n_cores = 8
    bpc = B // n_cores
    if _NC_CACHE[0] is None:
        _NC_CACHE[0] = _build(B_PER_CORE=bpc, T=T, NCH=NCH)
    nc = _NC_CACHE[0]
    in_maps = [{"x": xi} for xi in _gather_inputs(x)]
    trace = bool(os.environ.get("EMA_KERNEL_TRACE"))
    res = run_bass_kernel_spmd(nc, in_maps, core_ids=list(range(n_cores)),
                               trace=trace)
    if trace:
        kernel.last_result = res
    return _scatter_outputs([res.results[k] for k in range(n_cores)], x)


# revision 36
# speedup vs baseline: 1.1185x; 1.1185x over previous
"""AsymmetricEMA Trainium2 kernel (8 NeuronCores, Bass/Tile).

Reference recurrence: y_0 = x_0; y_t = a*y_{t-1} + (1-a)*x_t with
a = 0.99 if y_{t-1} > x_t else 0.5.  Equivalently (exactly):

    y_t = max(0.99*(y_{t-1}-x_t), 0.5*(y_{t-1}-x_t)) + x_t

**d-space formulation**: with d_t = y_t - x_t and delta_t = x_{t-1} - x_t
(precomputed on the host from the f32 input during the layout gather):

    d_t = max(0.99*u, 0.5*u),   u = d_{t-1} + delta_t ;   y_t = d_t + x_t

The device ships delta in, d out (same bytes as x/y), and the host adds x
back in f32 during the scatter.  The win: the DVE step body is 4 ALU ops
(add, mul, mul, max), which fits TWICE in the 8-stage DVE datapath -- so a
hand-written 2X_1PORT uop program (2 packed fp16/cycle) runs the whole
recurrence at 2 elem/cycle instead of 1 (validated exact on HW).

Time axis split into C=16 chunks of L=256 processed in parallel, each
warmed up with W=96 extra steps started from d := 0 at chunk_start - W
(contraction makes the warmed-up state accurate to ~1e-3 rel-norm).

Layout: host pre-gathers delta into round-major SBUF streaming order with
[C, G] column layout (chunk-major, group-minor) so the warmup chunk-shift
is -G elements (32B, 4B-aligned) and every DVE AP qualifies for 2x mode.
Each refill round is ONE contiguous 2 MiB DMA; drains are merged into one
DMA per contiguous Y-buffer run (3 DMAs for NBUFY=6), each targeting its
own partition-major-contiguous DRAM tensor (og0..og2).

DMA schedule: reads and writes are direction-phased (all refills precede
all drains on the single qSP HWDGE FIFO; refills that reuse a stream
buffer are emitted right after the body that frees it): measured per-core
HBM rates are 483 GB/s read-only, 397 GB/s write-only, but only ~371 GB/s
combined when directions interleave -- phase separation avoids the
read/write turnaround tax.  NBUFY=6 d-state rings let the DVE run ahead
of the trailing drain phase.

Sharding: batch (16) across 8 cores, 2 batches/core, pure data parallel.
"""
import os
import numpy as np
import orjson

# ---------------------------------------------------------------------------
# container workaround: this walrus build allows ONE sync-wait per
# instruction; hoist extras onto NoOps inserted before (same engine =>
# same order => identical sync semantics).
# ---------------------------------------------------------------------------
from concourse import bass as _bass

_MAX_WAITS = 1
_orig_to_json_bytes = _bass.Bass.to_json_bytes


def _split_waits_json(data: bytes) -> bytes:
    j = orjson.loads(data)
    n = [0]
    changed = False
    for fn in j.get("functions", []):
        for bb in fn.get("blocks", []):
            out = []
            for inst in bb.get("instructions", []):
                si = inst.get("sync_info")
                if si:
                    waits = si.get("on_wait") or []
                    if len(waits) > _MAX_WAITS:
                        changed = True
                        for w in waits[:-_MAX_WAITS]:
                            n[0] += 1
                            out.append({
                                "debug": inst.get("debug", 0),
                                "engine": inst["engine"],
                                "ins": [], "outs": [],
                                "name": f"I-waitsplit-{n[0]}",
                                "opcode": "NoOp",
                                "sync_info": {"on_update": [],
                                              "on_wait": [w]},
                            })
                        si["on_wait"] = waits[-_MAX_WAITS:]
                out.append(inst)
            bb["instructions"] = out
    return orjson.dumps(j) if changed else data


def _to_json_bytes_patched(self, *a, **k):
    return _split_waits_json(_orig_to_json_bytes(self, *a, **k))


_bass.Bass.to_json_bytes = _to_json_bytes_patched

from concourse import bass, mybir  # noqa: E402
from concourse.tile import TileContext  # noqa: E402
from concourse.bass_utils import run_bass_kernel_spmd  # noqa: E402

F16 = mybir.dt.float16
AF, AR = 0.99, 0.5

# ---------------------------------------------------------------------------
# custom DVE op: out = max((in0+in1)*C0, (in0+in1)*C1), with a hand-written
# 2X_1PORT uop program (el0 on datapath blocks 0-3, el1 on blocks 4-7).
# ---------------------------------------------------------------------------
_EMA_OP = [None]


def _get_ema_d_op():
    if _EMA_OP[0] is not None:
        return _EMA_OP[0]
    from concourse.dve_spec import Spec, Src0, Src1, C0, C1, maxx, lower
    from concourse.dve_uop import (DveOpSpec, UopConfig, AluOp, AluInp,
                                   DelayInp, InpSel, OutSel, OutPath,
                                   Trigger)
    from concourse import dve_ops
    from concourse.dve_ops import DveOp, OPS, _COMPILE_CACHE

    def _ref(in0, in1, s0, s1, imm2):
        u = (in0 + in1).astype(np.float32)
        return np.maximum(u * np.float32(s0), u * np.float32(s1)).astype(
            np.float32)

    u = Src0 + Src1
    spec = Spec(body=maxx(u * C0, u * C1), reference=_ref)
    uops_1x = lower(spec, ver="v3")

    A, D = AluInp, DelayInp
    u2 = UopConfig()
    u2.enable_input(InpSel.SRC_0, 1)      # chain0 at blk0
    u2.enable_input(InpSel.SRC_1, 2)      # chain1
    u2.enable_input(InpSel.CONST_0, 3)    # chain2
    u2.enable_input(InpSel.CONST_1, 4)    # chain3
    u2.enable_input(InpSel.SRC_0_HI, 5)   # chain4
    u2.enable_input(InpSel.SRC_1_HI, 6)   # chain5
    u2.require_inp0 = 1
    u2.require_inp1 = 1
    u2.trigger = (Trigger.SRC_TENSOR_DONE, Trigger.NONE, Trigger.NONE)
    b = u2.datapath_config
    # el0 (LO halves)
    b[0].enable_alu(AluOp.ADD, A.PREV_DELAY_0, A.PREV_DELAY_1)         # u0
    b[0].pass_through_delay(2, 3, 4, 5)
    b[1].enable_alu(AluOp.MULTIPLY, A.PREV_ALU_OUT, A.PREV_DELAY_2)    # m1_0
    b[1].enable_delay_from_src(D.PREV_ALU_OUT, 0)                      # u0
    b[1].pass_through_delay(2, 3, 4, 5)
    b[2].enable_alu(AluOp.MULTIPLY, A.PREV_DELAY_0, A.PREV_DELAY_3)    # m2_0
    b[2].enable_delay_from_src(D.PREV_ALU_OUT, 0)                      # m1_0
    b[2].pass_through_delay(2, 3, 4, 5)
    b[3].enable_alu(AluOp.MAX, A.PREV_DELAY_0, A.PREV_ALU_OUT)         # r0
    b[3].pass_through_delay(2, 3, 4, 5)
    # el1 (HI halves)
    b[4].enable_alu(AluOp.ADD, A.PREV_DELAY_4, A.PREV_DELAY_5)         # u1
    b[4].enable_delay_from_src(D.PREV_ALU_OUT, 0)                      # r0
    b[4].pass_through_delay(2, 3)
    b[5].enable_alu(AluOp.MULTIPLY, A.PREV_ALU_OUT, A.PREV_DELAY_2)    # m1_1
    b[5].enable_delay_from_src(D.PREV_ALU_OUT, 1)                      # u1
    b[5].pass_through_delay(0, 3)
    b[6].enable_alu(AluOp.MULTIPLY, A.PREV_DELAY_1, A.PREV_DELAY_3)    # m2_1
    b[6].enable_delay_from_src(D.PREV_ALU_OUT, 1)                      # m1_1
    b[6].pass_through_delay(0)
    b[7].enable_alu(AluOp.MAX, A.PREV_DELAY_1, A.PREV_ALU_OUT)         # r1
    b[7].pass_through_delay(0)
    u2.enable_output(OutSel.DELAY_0, OutPath.WR0_LO)   # r0
    u2.enable_output(OutSel.ALU_OUT, OutPath.WR0_HI)   # r1
    u2.validate("v3")

    name = "EMA_D2X_ANT"
    op = DveOp(name, spec, subdim=False, uops_sha={})
    if name not in dve_ops._SUB_OPCODE_FOR_NAME:
        OPS.append(op)
        dve_ops._SUB_OPCODE_FOR_NAME[name] = (
            dve_ops._CUSTOM_DVE_ROW_BASE + len(OPS) - 1)
    dve_ops.CUSTOM_DVE_SPECS[name] = spec
    row = dve_ops.get_dve_sub_opcode(name)
    compiled = DveOpSpec(name=name, opcode=row, uops=uops_1x,
                         uops_2x=[u2], rd1_en=True, perf_max=1)
    _COMPILE_CACHE[(name, "v3")] = compiled
    _EMA_OP[0] = op
    return op


# default kernel geometry (hardcoded for the 16x4096x1024 problem)
L, W, BLK = 256, 96, 32
NBUFY = int(os.environ.get("EMA_NBUFY", "6"))
NSTREAM_ENV = int(os.environ.get("EMA_NSTREAM", "3"))
DRAIN_MODE = os.environ.get("EMA_DRAIN", "late")  # late | interleave
# drain merge groups (rounds): maximal runs of rounds whose Y buffers
# (NP + q) % NBUFY are contiguous, so each group drains in ONE DMA.
def _dgroups(NR=8, NP=3, nbufy=None):
    nbufy = nbufy or NBUFY
    out, q0 = [], 0
    while q0 < NR:
        b0 = (NP + q0) % nbufy
        n = min(nbufy - b0, NR - q0)
        out.append((q0, q0 + n))
        q0 += n
    return out

DGROUPS = _dgroups()
if os.environ.get("EMA_SPLITD0", "0") == "1":
    # split the leading drain group so the write phase's first DMA only
    # waits on DVE round 0 (earlier leading edge)
    g0, g1 = DGROUPS[0]
    if g1 - g0 > 1:
        DGROUPS = [(g0, g0 + 1), (g0 + 1, g1)] + DGROUPS[1:]
if os.environ.get("EMA_SPLITDLAST", "0") == "1":
    # split the trailing drain group so the second-to-last round's write
    # starts as soon as ITS round completes (shorter critical-path tail)
    g0, g1 = DGROUPS[-1]
    if g1 - g0 > 1:
        DGROUPS = DGROUPS[:-1] + [(g0, g1 - 1), (g1 - 1, g1)]


# ---------------------------------------------------------------------------
# per-core SPMD program
# ---------------------------------------------------------------------------
def _build(B_PER_CORE=2, T=4096, NCH=1024, reps=1, mode="normal",
           bodies_per_rep=1):
    CBLK = NCH // 128          # channel blocks of 128 partitions
    G = B_PER_CORE * CBLK      # partition-groups (inner dim of a column)
    C = T // L                 # parallel time chunks (outer dim of a column)
    NR = L // BLK              # body column rounds (delta is read once)
    NP = W // BLK              # warmup rounds == pinned tail rounds
    NSTREAM = NSTREAM_ENV      # streaming delta buffers (rounds 0..NR-NP-1)
    NBUFX = NSTREAM + NP
    CW = G * C                 # one column's elements per partition
    FB = BLK * CW              # one buffer, flat
    assert W % BLK == 0 and L % BLK == 0 and NP + NSTREAM <= NR
    ema_op = _get_ema_d_op()

    nc = bass.Bass(num_devices=8)
    # x: round-major (each refill round = one fully contiguous 2 MiB DRAM
    # region -> maximal descriptors).  out: one DRAM tensor per drain merge
    # group, partition-major within the group, so each merged drain is one
    # DMA with large contiguous per-partition runs.
    x_ext = nc.declare_dram_parameter("x", [NR, 128, FB], F16,
                                      isOutput=False)
    og_ext = [nc.declare_dram_parameter(f"og{i}", [128, (g1 - g0) * FB],
                                        F16, isOutput=True)
              for i, (g0, g1) in enumerate(DGROUPS)]

    with TileContext(nc) as tc:
        with tc.tile_pool(name="rings", bufs=1) as rpool:
            # column layout [C, G]: chunk-major, group-minor, so the warmup
            # chunk-shift is -G elements (4B-aligned -> 2x eligible).
            X = rpool.tile([128, NBUFX, BLK, C, G], F16)   # delta buffers
            Y = rpool.tile([128, NBUFY, BLK, C, G], F16)   # d-state buffers
            XA = X.rearrange("p n a b c -> p (n a b c)")

            def xbuf(q):
                # body round q -> delta buffer index
                return q % NSTREAM if q < NR - NP else NSTREAM + (q - (NR - NP))

            def refill(q):
                nc.sync.dma_start(
                    out=X[:, xbuf(q)].rearrange("p a b c -> p (a b c)"),
                    in_=x_ext[q])

            deng = nc.scalar if os.environ.get("EMA_DRAINQ") == "act" \
                else nc.sync

            def drain_group(gi):
                # rounds g0..g1-1 from Y bufs (NP+g0)%NBUFY.. in ONE DMA
                g0, g1 = DGROUPS[gi]
                b0 = (NP + g0) % NBUFY
                assert b0 + (g1 - g0) <= NBUFY
                deng.dma_start(
                    out=og_ext[gi][:],
                    in_=Y[:, b0:b0 + (g1 - g0)].rearrange(
                        "p n a b c -> p (n a b c)"))

            def out_slice(qd):
                # round qd's slice of its group tensor (analysis modes)
                for gi, (g0, g1) in enumerate(DGROUPS):
                    if g0 <= qd < g1:
                        return og_ext[gi][:, (qd - g0) * FB:(qd - g0 + 1) * FB]
                raise AssertionError(qd)

            def drain(qd):
                deng.dma_start(
                    out=out_slice(qd),
                    in_=Y[:, (NP + qd) % NBUFY].rearrange(
                        "p a b c -> p (a b c)"))

            def flat(tile, q):
                return tile[:, q].rearrange("p a b c -> p (a b c)")

            def ema(out, in0, in1, s0=AF, s1=AR):
                binst = nc.vector._custom_dve(ema_op, out=out, in0=in0,
                                              in1=in1, s0=s0, s1=s1)
                binst.ins.perf_max = int(os.environ.get("EMA_PERF", "1"))
                return binst

            def ema0(out, src):
                # d := 0 (s0=s1=0 => max(0*u, 0*u) = 0; src just needs to be
                # a finite readable tile -- delta data, never junk SBUF).
                return ema(out, src, src, s0=0.0, s1=0.0)

            def warmup_buffer(wb):
                """Warmup columns [wb*BLK, wb*BLK+BLK): every chunk c reads
                chunk c-1's tail delta from pinned round NR-NP+wb via a
                G-element-shifted flat AP (chunk 0 lanes read junk; their
                state is re-initialized at the i==0 boundary)."""
                pb = NSTREAM + wb          # pinned delta buffer
                off = pb * FB              # flat offset of that buffer
                YF = flat(Y, wb % NBUFY)
                if mode == "nodep":
                    ema(YF[:, 0:FB], XA[:, off:off + FB], XA[:, off:off + FB])
                    return
                yb = wb % NBUFY
                if wb == 0:
                    # warmup start: d := 0 for every chunk, one instruction
                    ema0(Y[:, yb, 0], X[:, pb, 0])
                else:
                    ema(Y[:, yb, 0, 1:, :], Y[:, (wb - 1) % NBUFY, BLK - 1, 1:, :],
                        X[:, pb, 0, 0:C - 1, :])
                    ema0(Y[:, yb, 0, 0:1, :], X[:, pb, 0, 0:1, :])
                ema(YF[:, CW:FB], YF[:, 0:FB - CW],
                    XA[:, off + CW - G:off + FB - G])

            def body_buffer(qb):
                """Body columns [qb*BLK, qb*BLK+BLK) in two (three at the
                i==0 boundary) DVE instructions.  Instr B covers BLK-1
                columns in ONE instruction whose in0 is its own out shifted
                one column back -- elements stream in AP order, so each
                read lands CW elements after the write it depends on."""
                q = (NP + qb) % NBUFY
                YF, XF = flat(Y, q), flat(X, xbuf(qb))
                if mode == "nodep":
                    ema(YF[:, 0:FB], XF[:, 0:FB], XF[:, 0:FB])
                    return
                if qb == 0:
                    # i==0: chunks 1.. continue from warmup; chunk 0: d := 0
                    ema(Y[:, q, 0, 1:, :], Y[:, (NP - 1) % NBUFY, BLK - 1, 1:, :],
                        X[:, xbuf(0), 0, 1:, :])
                    ema0(Y[:, q, 0, 0:1, :], X[:, xbuf(0), 0, 0:1, :])
                else:
                    YP = flat(Y, (NP + qb - 1) % NBUFY)
                    ema(YF[:, 0:CW], YP[:, FB - CW:FB], XF[:, 0:CW])
                ema(YF[:, CW:FB], YF[:, 0:FB - CW], XF[:, CW:FB])

            def body():
                if mode == "dmaonly":
                    for q in range(NR):
                        refill(q)
                    return
                if mode in ("dmaio", "dmaio2"):
                    eng = nc.scalar if mode == "dmaio2" else nc.sync
                    for q in range(NR):
                        refill(q)
                        eng.dma_start(
                            out=out_slice(q),
                            in_=X[:, xbuf(q)].rearrange("p a b c -> p (a b c)"))
                    return
                if mode == "drainonly":
                    for q in range(NR):
                        nc.sync.dma_start(
                            out=out_slice(q),
                            in_=X[:, q % NBUFX].rearrange("p a b c -> p (a b c)"))
                    return
                if mode == "dmaphase":
                    for q in range(NR):
                        nc.sync.dma_start(
                            out=X[:, q % NSTREAM].rearrange("p a b c -> p (a b c)"),
                            in_=x_ext[q])
                    for q in range(NR):
                        nc.sync.dma_start(
                            out=out_slice(q),
                            in_=X[:, NSTREAM + q % NP].rearrange("p a b c -> p (a b c)"))
                    return
                if mode == "debug":
                    # X preloaded (below); run N warmups + M bodies, then
                    # drain warmup bufs to out rounds 0..NP-1 and body bufs
                    # to rounds NP..  (EMA_DBG="<nw>,<nb>")
                    nw, nb = map(int, os.environ.get("EMA_DBG", "3,0").split(","))
                    for wb in range(nw):
                        warmup_buffer(wb)
                    for qb in range(nb):
                        body_buffer(qb)
                    for wb in range(nw):
                        nc.sync.dma_start(
                            out=out_slice(wb),
                            in_=Y[:, wb % NBUFY].rearrange("p a b c -> p (a b c)"))
                    for qb in range(nb):
                        nc.sync.dma_start(
                            out=out_slice(NP + qb),
                            in_=Y[:, (NP + qb) % NBUFY].rearrange("p a b c -> p (a b c)"))
                    return
                do_dma = mode != "dveonly"
                do_refill = do_dma and mode != "norefill"
                do_drain = do_dma and mode != "nodrain"
                if do_refill:
                    # pinned tail first (warmup input), then the stream
                    # prefill.  Later stream refills are emitted right after
                    # the body that frees their buffer (program order defines
                    # both the Tile data binding and the qSP FIFO order).
                    for q in range(NR - NP, NR):
                        refill(q)
                    for q in range(NSTREAM):
                        refill(q)
                for wb in range(NP):
                    warmup_buffer(wb)
                # merged drains: group gi is emitted right after the body
                # whose round completes it (so its DVE wait can pass), and
                # always after all refill emissions (direction phasing on
                # the qSP FIFO).
                drain_after = {DGROUPS[gi][1] - 1: gi
                               for gi in range(len(DGROUPS))}
                for qb in range(NR):
                    body_buffer(qb)
                    if do_drain and DRAIN_MODE == "interleave":
                        drain(qb)
                    nq = NSTREAM + qb
                    if do_refill and nq < NR - NP:
                        refill(nq)
                    if do_drain and DRAIN_MODE == "late" and qb in drain_after:
                        drain_group(drain_after[qb])

            if mode in ("dveonly", "drainonly", "dmaphase", "debug", "norefill"):
                for q in range(NBUFX):
                    rnd = q if q < NSTREAM else NR - NP + (q - NSTREAM)
                    nc.sync.dma_start(
                        out=X[:, q].rearrange("p a b c -> p (a b c)"),
                        in_=x_ext[rnd])
            if reps == 1 and bodies_per_rep > 1:
                # unrolled replication with a cross-core barrier after each
                # body: keeps the 8 cores phase-aligned (as in a real
                # single-shot launch) for faithful per-body timing.
                # EMA_NBARRIER=2 doubles the barrier for barrier-cost calib.
                nb = int(os.environ.get("EMA_NBARRIER", "0"))
                for _ in range(bodies_per_rep):
                    body()
                    for _ in range(nb):
                        nc.all_core_barrier()
            elif reps == 1:
                body()
            else:
                # timing-only replication: repeat the identical program in a
                # hardware loop so exec time ~= reps * body.  NOTE: across
                # reps the 8 cores drift out of phase, so their read/write
                # DMA phases mix at the HBM level; the per-body estimate is
                # therefore a CONSERVATIVE (upper) bound on the single-shot
                # time, where all cores launch in lockstep and the
                # direction-phased DMA schedule avoids r/w turnaround.
                with tc.For_i(0, reps):
                    for _ in range(bodies_per_rep):
                        body()

    mybir.codegen_inst_isa_subclasses(nc)
    return nc


_NC_CACHE = [None]


def _gather_inputs(x):
    """x: [B, T, NCH] f32 -> list of per-core round-major fp16 delta arrays.

    delta_t = x_{t-1} - x_t (delta_0 = 0, never read).  Round-major with
    [C, G] columns: xdev[q][p][j][c][(b,k)] = delta[b, k*128+p, c*L + q*BLK + j].
    """
    B, T, NCH = x.shape
    n_cores, bpc = 8, B // 8
    CBLK = NCH // 128
    C = T // L
    delta = np.empty_like(x)
    delta[:, 0] = 0.0
    np.subtract(x[:, :-1], x[:, 1:], out=delta[:, 1:])
    dt_ = np.swapaxes(delta, 1, 2).astype(np.float16)   # [B, NCH, T]
    xg = dt_.reshape(B, CBLK, 128, C, L // BLK, BLK)
    # xg: [b, k, p, c, q, j] -> [q, p, j, c, b, k] (round-major DRAM)
    xg = np.ascontiguousarray(xg.transpose(4, 2, 5, 3, 0, 1))
    ins = []
    for kc in range(n_cores):
        blk = xg[:, :, :, :, bpc * kc:bpc * (kc + 1)]  # [NR,128,BLK,C,bpc,CBLK]
        ins.append(np.ascontiguousarray(
            blk.reshape(blk.shape[0], 128, -1)))
    return ins


def _scatter_outputs(outs, x):
    """outs: per-core dict of drain-group arrays og{i} [128, ng*FB] fp16
    d-values -> y = d + x f32."""
    B, T, NCH = x.shape
    n_cores, bpc = 8, B // 8
    CBLK = NCH // 128
    C = T // L
    NRD = L // BLK
    y = np.empty((B, NCH, T), np.float32)
    for kc in range(n_cores):
        # reassemble [NRD, 128, BLK, C, bpc, CBLK] from group tensors
        parts = []
        for gi, (g0, g1) in enumerate(DGROUPS):
            a = outs[kc][f"og{gi}"].reshape(128, g1 - g0, BLK, C, bpc, CBLK)
            parts.append(a.transpose(1, 0, 2, 3, 4, 5))
        o = np.concatenate(parts, axis=0)
        # -> [bpc, CBLK, 128, C, NRD, BLK]
        o = o.transpose(4, 5, 1, 3, 0, 2).astype(np.float32)
        o = o.reshape(bpc, NCH, C, L)
        y[bpc * kc:bpc * (kc + 1)] = o.reshape(bpc, NCH, T)
    yout = np.ascontiguousarray(np.swapaxes(y, 1, 2))
    yout += x
    return yout


def kernel(x: np.ndarray) -> np.ndarray:
    x = np.asarray(x, dtype=np.float32)
    B, T, NCH = x.shape  # (16, 4096, 1024)
    n_cores = 8
    bpc = B // n_cores
    if _NC_CACHE[0] is None:
        _NC_CACHE[0] = _build(B_PER_CORE=bpc, T=T, NCH=NCH)
    nc = _NC_CACHE[0]
    in_maps = [{"x": xi} for xi in _gather_inputs(x)]
    trace = bool(os.environ.get("EMA_KERNEL_TRACE"))
    res = run_bass_kernel_spmd(nc, in_maps, core_ids=list(range(n_cores)),
                               trace=trace)
    if trace:
        kernel.last_result = res
    return _scatter_outputs([res.results[k] for k in range(n_cores)], x)


# revision 40
# speedup vs baseline: 1.1452x; 1.0238x over previous
"""AsymmetricEMA Trainium2 kernel (8 NeuronCores, Bass/Tile).

Reference recurrence: y_0 = x_0; y_t = a*y_{t-1} + (1-a)*x_t with
a = 0.99 if y_{t-1} > x_t else 0.5.  Equivalently (exactly):

    y_t = max(0.99*(y_{t-1}-x_t), 0.5*(y_{t-1}-x_t)) + x_t

**d-space formulation**: with d_t = y_t - x_t and delta_t = x_{t-1} - x_t
(precomputed on the host from the f32 input during the layout gather):

    d_t = max(0.99*u, 0.5*u),   u = d_{t-1} + delta_t ;   y_t = d_t + x_t

The device ships delta in, d out (same bytes as x/y), and the host adds x
back in f32 during the scatter.  The win: the DVE step body is 4 ALU ops
(add, mul, mul, max), which fits TWICE in the 8-stage DVE datapath -- so a
hand-written 2X_1PORT uop program (2 packed fp16/cycle) runs the whole
recurrence at 2 elem/cycle instead of 1 (validated exact on HW).

Time axis split into C=16 chunks of L=256 processed in parallel, each
warmed up with W=96 extra steps started from d := 0 at chunk_start - W
(contraction makes the warmed-up state accurate to ~1e-3 rel-norm).

Layout: host pre-gathers delta into round-major SBUF streaming order with
[C, G] column layout (chunk-major, group-minor) so the warmup chunk-shift
is -G elements (32B, 4B-aligned) and every DVE AP qualifies for 2x mode.
Each refill round is ONE contiguous 2 MiB DMA; drains are merged into one
DMA per contiguous Y-buffer run (3 DMAs for NBUFY=6), each targeting its
own partition-major-contiguous DRAM tensor (og0..og2).

DMA schedule: reads and writes are direction-phased (all refills precede
all drains on the single qSP HWDGE FIFO; refills that reuse a stream
buffer are emitted right after the body that frees it): measured per-core
HBM rates are 483 GB/s read-only, 397 GB/s write-only, but only ~371 GB/s
combined when directions interleave -- phase separation avoids the
read/write turnaround tax.  NBUFY=6 d-state rings let the DVE run ahead
of the trailing drain phase.

Sharding: batch (16) across 8 cores, 2 batches/core, pure data parallel.
"""
import os
import numpy as np
import orjson

# ---------------------------------------------------------------------------
# container workaround: this walrus build allows ONE sync-wait per
# instruction; hoist extras onto NoOps inserted before (same engine =>
# same order => identical sync semantics).
# ---------------------------------------------------------------------------
from concourse import bass as _bass

_MAX_WAITS = 1
_orig_to_json_bytes = _bass.Bass.to_json_bytes


def _split_waits_json(data: bytes) -> bytes:
    j = orjson.loads(data)
    n = [0]
    changed = False
    for fn in j.get("functions", []):
        for bb in fn.get("blocks", []):
            out = []
            for inst in bb.get("instructions", []):
                si = inst.get("sync_info")
                if si:
                    waits = si.get("on_wait") or []
                    if len(waits) > _MAX_WAITS:
                        changed = True
                        for w in waits[:-_MAX_WAITS]:
                            n[0] += 1
                            out.append({
                                "debug": inst.get("debug", 0),
                                "engine": inst["engine"],
                                "ins": [], "outs": [],
                                "name": f"I-waitsplit-{n[0]}",
                                "opcode": "NoOp",
                                "sync_info": {"on_update": [],
                                              "on_wait": [w]},
                            })
                        si["on_wait"] = waits[-_MAX_WAITS:]
                out.append(inst)
            bb["instructions"] = out
    return orjson.dumps(j) if changed else data


def _to_json_bytes_patched(self, *a, **k):
    return _split_waits_json(_orig_to_json_bytes(self, *a, **k))


_bass.Bass.to_json_bytes = _to_json_bytes_patched

from concourse import bass, mybir  # noqa: E402
from concourse.tile import TileContext  # noqa: E402
from concourse.bass_utils import run_bass_kernel_spmd  # noqa: E402

F16 = mybir.dt.float16
AF, AR = 0.99, 0.5

# ---------------------------------------------------------------------------
# custom DVE op: out = max((in0+in1)*C0, (in0+in1)*C1), with a hand-written
# 2X_1PORT uop program (el0 on datapath blocks 0-3, el1 on blocks 4-7).
# ---------------------------------------------------------------------------
_EMA_OP = [None]


def _get_ema_d_op():
    if _EMA_OP[0] is not None:
        return _EMA_OP[0]
    from concourse.dve_spec import Spec, Src0, Src1, C0, C1, maxx, lower
    from concourse.dve_uop import (DveOpSpec, UopConfig, AluOp, AluInp,
                                   DelayInp, InpSel, OutSel, OutPath,
                                   Trigger)
    from concourse import dve_ops
    from concourse.dve_ops import DveOp, OPS, _COMPILE_CACHE

    def _ref(in0, in1, s0, s1, imm2):
        u = (in0 + in1).astype(np.float32)
        return np.maximum(u * np.float32(s0), u * np.float32(s1)).astype(
            np.float32)

    u = Src0 + Src1
    spec = Spec(body=maxx(u * C0, u * C1), reference=_ref)
    uops_1x = lower(spec, ver="v3")

    A, D = AluInp, DelayInp
    u2 = UopConfig()
    u2.enable_input(InpSel.SRC_0, 1)      # chain0 at blk0
    u2.enable_input(InpSel.SRC_1, 2)      # chain1
    u2.enable_input(InpSel.CONST_0, 3)    # chain2
    u2.enable_input(InpSel.CONST_1, 4)    # chain3
    u2.enable_input(InpSel.SRC_0_HI, 5)   # chain4
    u2.enable_input(InpSel.SRC_1_HI, 6)   # chain5
    u2.require_inp0 = 1
    u2.require_inp1 = 1
    u2.trigger = (Trigger.SRC_TENSOR_DONE, Trigger.NONE, Trigger.NONE)
    b = u2.datapath_config
    # el0 (LO halves)
    b[0].enable_alu(AluOp.ADD, A.PREV_DELAY_0, A.PREV_DELAY_1)         # u0
    b[0].pass_through_delay(2, 3, 4, 5)
    b[1].enable_alu(AluOp.MULTIPLY, A.PREV_ALU_OUT, A.PREV_DELAY_2)    # m1_0
    b[1].enable_delay_from_src(D.PREV_ALU_OUT, 0)                      # u0
    b[1].pass_through_delay(2, 3, 4, 5)
    b[2].enable_alu(AluOp.MULTIPLY, A.PREV_DELAY_0, A.PREV_DELAY_3)    # m2_0
    b[2].enable_delay_from_src(D.PREV_ALU_OUT, 0)                      # m1_0
    b[2].pass_through_delay(2, 3, 4, 5)
    b[3].enable_alu(AluOp.MAX, A.PREV_DELAY_0, A.PREV_ALU_OUT)         # r0
    b[3].pass_through_delay(2, 3, 4, 5)
    # el1 (HI halves)
    b[4].enable_alu(AluOp.ADD, A.PREV_DELAY_4, A.PREV_DELAY_5)         # u1
    b[4].enable_delay_from_src(D.PREV_ALU_OUT, 0)                      # r0
    b[4].pass_through_delay(2, 3)
    b[5].enable_alu(AluOp.MULTIPLY, A.PREV_ALU_OUT, A.PREV_DELAY_2)    # m1_1
    b[5].enable_delay_from_src(D.PREV_ALU_OUT, 1)                      # u1
    b[5].pass_through_delay(0, 3)
    b[6].enable_alu(AluOp.MULTIPLY, A.PREV_DELAY_1, A.PREV_DELAY_3)    # m2_1
    b[6].enable_delay_from_src(D.PREV_ALU_OUT, 1)                      # m1_1
    b[6].pass_through_delay(0)
    b[7].enable_alu(AluOp.MAX, A.PREV_DELAY_1, A.PREV_ALU_OUT)         # r1
    b[7].pass_through_delay(0)
    u2.enable_output(OutSel.DELAY_0, OutPath.WR0_LO)   # r0
    u2.enable_output(OutSel.ALU_OUT, OutPath.WR0_HI)   # r1
    u2.validate("v3")

    name = "EMA_D2X_ANT"
    op = DveOp(name, spec, subdim=False, uops_sha={})
    if name not in dve_ops._SUB_OPCODE_FOR_NAME:
        OPS.append(op)
        dve_ops._SUB_OPCODE_FOR_NAME[name] = (
            dve_ops._CUSTOM_DVE_ROW_BASE + len(OPS) - 1)
    dve_ops.CUSTOM_DVE_SPECS[name] = spec
    row = dve_ops.get_dve_sub_opcode(name)
    compiled = DveOpSpec(name=name, opcode=row, uops=uops_1x,
                         uops_2x=[u2], rd1_en=True, perf_max=1)
    _COMPILE_CACHE[(name, "v3")] = compiled
    _EMA_OP[0] = op
    return op


# default kernel geometry (hardcoded for the 16x4096x1024 problem)
L, W, BLK = 256, 96, 32
NBUFY = int(os.environ.get("EMA_NBUFY", "6"))
NSTREAM_ENV = int(os.environ.get("EMA_NSTREAM", "3"))
DRAIN_MODE = os.environ.get("EMA_DRAIN", "late")  # late | interleave
# drain merge groups (rounds): maximal runs of rounds whose Y buffers
# (NP + q) % NBUFY are contiguous, so each group drains in ONE DMA.
def _dgroups(NR=8, NP=3, nbufy=None):
    nbufy = nbufy or NBUFY
    out, q0 = [], 0
    while q0 < NR:
        b0 = (NP + q0) % nbufy
        n = min(nbufy - b0, NR - q0)
        out.append((q0, q0 + n))
        q0 += n
    return out

DGROUPS = _dgroups()
if os.environ.get("EMA_SPLITD0", "0") == "1":
    # split the leading drain group so the write phase's first DMA only
    # waits on DVE round 0 (earlier leading edge)
    g0, g1 = DGROUPS[0]
    if g1 - g0 > 1:
        DGROUPS = [(g0, g0 + 1), (g0 + 1, g1)] + DGROUPS[1:]
if os.environ.get("EMA_SPLITDLAST", "0") == "1":
    # split the trailing drain group so the second-to-last round's write
    # starts as soon as ITS round completes (shorter critical-path tail)
    g0, g1 = DGROUPS[-1]
    if g1 - g0 > 1:
        DGROUPS = DGROUPS[:-1] + [(g0, g1 - 1), (g1 - 1, g1)]


# ---------------------------------------------------------------------------
# per-core SPMD program
# ---------------------------------------------------------------------------
def _build(B_PER_CORE=2, T=4096, NCH=1024, reps=1, mode="normal",
           bodies_per_rep=1):
    CBLK = NCH // 128          # channel blocks of 128 partitions
    G = B_PER_CORE * CBLK      # partition-groups (inner dim of a column)
    C = T // L                 # parallel time chunks (outer dim of a column)
    NR = L // BLK              # body column rounds (delta is read once)
    NP = W // BLK              # warmup rounds == pinned tail rounds
    NSTREAM = NSTREAM_ENV      # streaming delta buffers (rounds 0..NR-NP-1)
    NBUFX = NSTREAM + NP
    CW = G * C                 # one column's elements per partition
    FB = BLK * CW              # one buffer, flat
    assert W % BLK == 0 and L % BLK == 0 and NP + NSTREAM <= NR
    ema_op = _get_ema_d_op()

    nc = bass.Bass(num_devices=8)
    # x: round-major (each refill round = one fully contiguous 2 MiB DRAM
    # region -> maximal descriptors).  out: one DRAM tensor per drain merge
    # group, partition-major within the group, so each merged drain is one
    # DMA with large contiguous per-partition runs.
    x_ext = nc.declare_dram_parameter("x", [NR, 128, FB], F16,
                                      isOutput=False)
    og_ext = [nc.declare_dram_parameter(f"og{i}", [128, (g1 - g0) * FB],
                                        F16, isOutput=True)
              for i, (g0, g1) in enumerate(DGROUPS)]

    with TileContext(nc) as tc:
        with tc.tile_pool(name="rings", bufs=1) as rpool:
            # column layout [C, G]: chunk-major, group-minor, so the warmup
            # chunk-shift is -G elements (4B-aligned -> 2x eligible).
            X = rpool.tile([128, NBUFX, BLK, C, G], F16)   # delta buffers
            Y = rpool.tile([128, NBUFY, BLK, C, G], F16)   # d-state buffers
            XA = X.rearrange("p n a b c -> p (n a b c)")

            def xbuf(q):
                # body round q -> delta buffer index
                return q % NSTREAM if q < NR - NP else NSTREAM + (q - (NR - NP))

            def refill(q):
                nc.sync.dma_start(
                    out=X[:, xbuf(q)].rearrange("p a b c -> p (a b c)"),
                    in_=x_ext[q])

            deng = nc.scalar if os.environ.get("EMA_DRAINQ") == "act" \
                else nc.sync

            def drain_group(gi):
                # rounds g0..g1-1 from Y bufs (NP+g0)%NBUFY.. in ONE DMA
                g0, g1 = DGROUPS[gi]
                b0 = (NP + g0) % NBUFY
                assert b0 + (g1 - g0) <= NBUFY
                deng.dma_start(
                    out=og_ext[gi][:],
                    in_=Y[:, b0:b0 + (g1 - g0)].rearrange(
                        "p n a b c -> p (n a b c)"))

            def out_slice(qd):
                # round qd's slice of its group tensor (analysis modes)
                for gi, (g0, g1) in enumerate(DGROUPS):
                    if g0 <= qd < g1:
                        return og_ext[gi][:, (qd - g0) * FB:(qd - g0 + 1) * FB]
                raise AssertionError(qd)

            def drain(qd):
                deng.dma_start(
                    out=out_slice(qd),
                    in_=Y[:, (NP + qd) % NBUFY].rearrange(
                        "p a b c -> p (a b c)"))

            def flat(tile, q):
                return tile[:, q].rearrange("p a b c -> p (a b c)")

            def ema(out, in0, in1, s0=AF, s1=AR):
                binst = nc.vector._custom_dve(ema_op, out=out, in0=in0,
                                              in1=in1, s0=s0, s1=s1)
                binst.ins.perf_max = int(os.environ.get("EMA_PERF", "1"))
                return binst

            def ema0(out, src):
                # d := 0 (s0=s1=0 => max(0*u, 0*u) = 0; src just needs to be
                # a finite readable tile -- delta data, never junk SBUF).
                return ema(out, src, src, s0=0.0, s1=0.0)

            def warmup_buffer(wb):
                """Warmup columns [wb*BLK, wb*BLK+BLK): every chunk c reads
                chunk c-1's tail delta from pinned round NR-NP+wb via a
                G-element-shifted flat AP (chunk 0 lanes read junk; their
                state is re-initialized at the i==0 boundary)."""
                pb = NSTREAM + wb          # pinned delta buffer
                off = pb * FB              # flat offset of that buffer
                YF = flat(Y, wb % NBUFY)
                if mode == "nodep":
                    ema(YF[:, 0:FB], XA[:, off:off + FB], XA[:, off:off + FB])
                    return
                yb = wb % NBUFY
                if wb == 0:
                    # warmup start: d := 0 for every chunk, one instruction
                    ema0(Y[:, yb, 0], X[:, pb, 0])
                else:
                    ema(Y[:, yb, 0, 1:, :], Y[:, (wb - 1) % NBUFY, BLK - 1, 1:, :],
                        X[:, pb, 0, 0:C - 1, :])
                    ema0(Y[:, yb, 0, 0:1, :], X[:, pb, 0, 0:1, :])
                ema(YF[:, CW:FB], YF[:, 0:FB - CW],
                    XA[:, off + CW - G:off + FB - G])

            def body_buffer(qb):
                """Body columns [qb*BLK, qb*BLK+BLK) in two (three at the
                i==0 boundary) DVE instructions.  Instr B covers BLK-1
                columns in ONE instruction whose in0 is its own out shifted
                one column back -- elements stream in AP order, so each
                read lands CW elements after the write it depends on."""
                q = (NP + qb) % NBUFY
                YF, XF = flat(Y, q), flat(X, xbuf(qb))
                if mode == "nodep":
                    ema(YF[:, 0:FB], XF[:, 0:FB], XF[:, 0:FB])
                    return
                if qb == 0:
                    # i==0: chunks 1.. continue from warmup; chunk 0: d := 0
                    ema(Y[:, q, 0, 1:, :], Y[:, (NP - 1) % NBUFY, BLK - 1, 1:, :],
                        X[:, xbuf(0), 0, 1:, :])
                    ema0(Y[:, q, 0, 0:1, :], X[:, xbuf(0), 0, 0:1, :])
                else:
                    YP = flat(Y, (NP + qb - 1) % NBUFY)
                    ema(YF[:, 0:CW], YP[:, FB - CW:FB], XF[:, 0:CW])
                ema(YF[:, CW:FB], YF[:, 0:FB - CW], XF[:, CW:FB])

            def body():
                if mode == "dmaonly":
                    for q in range(NR):
                        refill(q)
                    return
                if mode in ("dmaio", "dmaio2"):
                    eng = nc.scalar if mode == "dmaio2" else nc.sync
                    for q in range(NR):
                        refill(q)
                        eng.dma_start(
                            out=out_slice(q),
                            in_=X[:, xbuf(q)].rearrange("p a b c -> p (a b c)"))
                    return
                if mode == "drainonly":
                    for q in range(NR):
                        nc.sync.dma_start(
                            out=out_slice(q),
                            in_=X[:, q % NBUFX].rearrange("p a b c -> p (a b c)"))
                    return
                if mode == "dmaphase":
                    for q in range(NR):
                        nc.sync.dma_start(
                            out=X[:, q % NSTREAM].rearrange("p a b c -> p (a b c)"),
                            in_=x_ext[q])
                    for q in range(NR):
                        nc.sync.dma_start(
                            out=out_slice(q),
                            in_=X[:, NSTREAM + q % NP].rearrange("p a b c -> p (a b c)"))
                    return
                if mode == "debug":
                    # X preloaded (below); run N warmups + M bodies, then
                    # drain warmup bufs to out rounds 0..NP-1 and body bufs
                    # to rounds NP..  (EMA_DBG="<nw>,<nb>")
                    nw, nb = map(int, os.environ.get("EMA_DBG", "3,0").split(","))
                    for wb in range(nw):
                        warmup_buffer(wb)
                    for qb in range(nb):
                        body_buffer(qb)
                    for wb in range(nw):
                        nc.sync.dma_start(
                            out=out_slice(wb),
                            in_=Y[:, wb % NBUFY].rearrange("p a b c -> p (a b c)"))
                    for qb in range(nb):
                        nc.sync.dma_start(
                            out=out_slice(NP + qb),
                            in_=Y[:, (NP + qb) % NBUFY].rearrange("p a b c -> p (a b c)"))
                    return
                do_dma = mode != "dveonly"
                do_refill = do_dma and mode != "norefill"
                do_drain = do_dma and mode != "nodrain"
                if do_refill:
                    # pinned tail first (warmup input), then the stream
                    # prefill.  Later stream refills are emitted right after
                    # the body that frees their buffer (program order defines
                    # both the Tile data binding and the qSP FIFO order).
                    for q in range(NR - NP, NR):
                        refill(q)
                    for q in range(NSTREAM):
                        refill(q)
                for wb in range(NP):
                    warmup_buffer(wb)
                # merged drains: group gi is emitted right after the body
                # whose round completes it (so its DVE wait can pass), and
                # always after all refill emissions (direction phasing on
                # the qSP FIFO).
                drain_after = {DGROUPS[gi][1] - 1: gi
                               for gi in range(len(DGROUPS))}
                for qb in range(NR):
                    body_buffer(qb)
                    if do_drain and DRAIN_MODE == "interleave":
                        drain(qb)
                    nq = NSTREAM + qb
                    if do_refill and nq < NR - NP:
                        refill(nq)
                    if do_drain and DRAIN_MODE == "late" and qb in drain_after:
                        drain_group(drain_after[qb])

            if mode in ("dveonly", "drainonly", "dmaphase", "debug", "norefill"):
                for q in range(NBUFX):
                    rnd = q if q < NSTREAM else NR - NP + (q - NSTREAM)
                    nc.sync.dma_start(
                        out=X[:, q].rearrange("p a b c -> p (a b c)"),
                        in_=x_ext[rnd])
            if reps == 1 and bodies_per_rep > 1:
                # unrolled replication with a cross-core barrier after each
                # body: keeps the 8 cores phase-aligned (as in a real
                # single-shot launch) for faithful per-body timing.
                # EMA_NBARRIER=2 doubles the barrier for barrier-cost calib.
                nb = int(os.environ.get("EMA_NBARRIER", "0"))
                for _ in range(bodies_per_rep):
                    body()
                    for _ in range(nb):
                        nc.all_core_barrier()
            elif reps == 1:
                body()
            else:
                # timing-only replication: repeat the identical program in a
                # hardware loop so exec time ~= reps * body.  NOTE: across
                # reps the 8 cores drift out of phase, so their read/write
                # DMA phases mix at the HBM level; the per-body estimate is
                # therefore a CONSERVATIVE (upper) bound on the single-shot
                # time, where all cores launch in lockstep and the
                # direction-phased DMA schedule avoids r/w turnaround.
                with tc.For_i(0, reps):
                    for _ in range(bodies_per_rep):
                        body()

    mybir.codegen_inst_isa_subclasses(nc)
    return nc


_NC_CACHE = [None]
_RUNNER_CACHE = [None]


def _fast_runner(nc, n_cores=8):
    """Build (once) a cached jitted dispatcher for repeat kernel() calls --
    run_bass_kernel_spmd re-traces jax on every call (~10 s); this reuses
    one compiled executable.  Same bass2jax lowering, no donation (outputs
    are fully written by the drains)."""
    if _RUNNER_CACHE[0] is not None:
        return _RUNNER_CACHE[0]
    import jax
    from jax.sharding import Mesh, PartitionSpec
    from jax.experimental.shard_map import shard_map
    from concourse.bass2jax import (_bass_exec_p, install_neuronx_cc_hook,
                                    partition_id_tensor)
    install_neuronx_cc_hook()
    partition_name = (nc.partition_id_tensor.name
                      if nc.partition_id_tensor else None)
    in_names, out_names, out_avals = [], [], []
    for alloc in nc.m.functions[0].allocations:
        if not isinstance(alloc, mybir.MemoryLocationSet):
            continue
        name = alloc.memorylocations[0].name
        if alloc.kind == "ExternalInput":
            if name != partition_name:
                in_names.append(name)
        elif alloc.kind == "ExternalOutput":
            out_names.append(name)
            out_avals.append(jax.core.ShapedArray(
                tuple(alloc.tensor_shape), mybir.dt.np(alloc.dtype)))
    all_in = list(in_names) + list(out_names)
    if partition_name is not None:
        all_in.append(partition_name)

    def _body(*args):
        operands = list(args)
        if partition_name is not None:
            operands.append(partition_id_tensor())
        return tuple(_bass_exec_p.bind(
            *operands, out_avals=tuple(out_avals), in_names=tuple(all_in),
            out_names=tuple(out_names),
            lowering_input_output_aliases=(),
            sim_require_finite=True, sim_require_nnan=True, nc=nc))

    devices = jax.devices()[:n_cores]
    mesh = Mesh(np.asarray(devices), ("core",))
    nspec = len(in_names) + len(out_names)
    sharded = jax.jit(
        shard_map(_body, mesh=mesh,
                  in_specs=(PartitionSpec("core"),) * nspec,
                  out_specs=(PartitionSpec("core"),) * len(out_names),
                  check_rep=False),
        keep_unused=True)
    zeros = [np.zeros((n_cores * a.shape[0], *a.shape[1:]), a.dtype)
             for a in out_avals]
    _RUNNER_CACHE[0] = (sharded, in_names, out_names, out_avals, zeros)
    return _RUNNER_CACHE[0]


def _gather_inputs(x):
    """x: [B, T, NCH] f32 -> list of per-core round-major fp16 delta arrays.

    delta_t = x_{t-1} - x_t (delta_0 = 0, never read).  Round-major with
    [C, G] columns: xdev[q][p][j][c][(b,k)] = delta[b, k*128+p, c*L + q*BLK + j].
    One strided copy per core (cast and subtract run on contiguous arrays).
    """
    B, T, NCH = x.shape
    n_cores, bpc = 8, B // 8
    CBLK = NCH // 128
    C = T // L
    delta = np.empty_like(x)
    delta[:, 0] = 0.0
    np.subtract(x[:, :-1], x[:, 1:], out=delta[:, 1:])
    dh = delta.astype(np.float16)              # contiguous cast (fast)
    xg = np.swapaxes(dh, 1, 2).reshape(B, CBLK, 128, C, L // BLK, BLK)
    xg = xg.transpose(4, 2, 5, 3, 0, 1)        # view [NR,128,BLK,C,B,CBLK]
    ins = []
    for kc in range(n_cores):
        blk = np.ascontiguousarray(xg[:, :, :, :, bpc * kc:bpc * (kc + 1)])
        ins.append(blk.reshape(blk.shape[0], 128, -1))
    return ins


def _scatter_outputs(outs, x):
    """outs: per-core dict of drain-group arrays og{i} [128, ng*FB] fp16
    d-values -> y = d + x f32 (built as x.copy() += d, one strided pass)."""
    B, T, NCH = x.shape
    n_cores, bpc = 8, B // 8
    CBLK = NCH // 128
    C = T // L
    yout = x.copy()
    for kc in range(n_cores):
        parts = [outs[kc][f"og{gi}"]
                 .reshape(128, g1 - g0, BLK, C, bpc, CBLK)
                 .transpose(1, 0, 2, 3, 4, 5)
                 for gi, (g0, g1) in enumerate(DGROUPS)]
        o = np.concatenate(parts, axis=0).astype(np.float32)
        # o: [NRD, 128, BLK, C, bpc, CBLK] -> [bpc, C, NRD, BLK, CBLK, 128]
        o = o.transpose(4, 3, 0, 2, 5, 1).reshape(bpc, T, NCH)
        yout[bpc * kc:bpc * (kc + 1)] += o
    return yout


def kernel(x: np.ndarray) -> np.ndarray:
    x = np.asarray(x, dtype=np.float32)
    if not hasattr(kernel, "_warm"):
        kernel._warm = False
    B, T, NCH = x.shape  # (16, 4096, 1024)
    n_cores = 8
    bpc = B // n_cores
    if _NC_CACHE[0] is None:
        _NC_CACHE[0] = _build(B_PER_CORE=bpc, T=T, NCH=NCH)
    nc = _NC_CACHE[0]
    ins = _gather_inputs(x)
    trace = bool(os.environ.get("EMA_KERNEL_TRACE"))
    if trace or not kernel._warm:
        # first call / trace path: the standard run_bass_kernel_spmd route
        in_maps = [{"x": xi} for xi in ins]
        res = run_bass_kernel_spmd(nc, in_maps,
                                   core_ids=list(range(n_cores)),
                                   trace=trace)
        if trace:
            kernel.last_result = res
        kernel._warm = True
        return _scatter_outputs([res.results[k] for k in range(n_cores)], x)
    # repeat calls: cached jitted dispatcher (identical lowering, no re-jit)
    sharded, in_names, out_names, out_avals, zeros = _fast_runner(nc)
    gin = [np.concatenate(ins, axis=0)]
    out_arrs = sharded(*gin, *zeros)
    outs = []
    for kc in range(n_cores):
        outs.append({name: np.asarray(out_arrs[i]).reshape(
            n_cores, *out_avals[i].shape)[kc]
            for i, name in enumerate(out_names)})
    return _scatter_outputs(outs, x)
